# revision 16
# baseline (speedup 1.0000x reference)
"""CRF output layer (loss + viterbi pred + rearranged log_probs) on 8 Trainium2 cores.

Device (per core, 16 of 128 batch rows):
  - Viterbi value recursion: per step, PE broadcasts each row's alpha across
    partitions (ones-matmul into PSUM); a custom fused DVE op computes
    max_i(alpha_i + M_ij) per j in one pass (bit-exact fp32). Alphas for every
    step stream to HBM.
  - Forward (logsumexp) recursion in exp space: P = exp(alpha - rowmax) via ACT,
    fp32 matmul with exp(M) on PE, Ln + add-back on ACT/DVE.
  - log-softmax denominators + per-token argmax/max of logits.
Host: gold-path score, backtrace from stored alphas (bit-exact, lowest-index
tie-break), loss assembly, logit swap + denominator subtraction.
"""
import numpy as np

B, T, K = 128, 512, 256
NCORES = 8
BC = B // NCORES  # 16 rows per core

_CACHE = {}
LAST_SPMD_WALL = None


# ---------------------------------------------------------------------------
# custom DVE op: out = in0 + in1 ; accum_out = max over free dim (seed -FLT_MAX)
# ---------------------------------------------------------------------------
def _get_add_max_op():
    import concourse.dve_ops as dve_ops
    from concourse.dve_spec import Spec, Src0, Src1, maxx, lower, _has_src1
    from concourse.dve_uop import DveOpSpec

    name = "ANT_ADD_MAX_REDUCE_X"
    for op in dve_ops.OPS:
        if op.name == name:
            return op
    spec = Spec(body=Src0 + Src1, accum=maxx)
    row = dve_ops._CUSTOM_DVE_ROW_BASE + len(dve_ops.OPS)
    assert row < 0x20
    dve_ops._SUB_OPCODE_FOR_NAME[name] = row
    shas = {}
    for ver in ("v3", "v4"):
        uops = lower(spec, ver=ver)
        shas[ver] = DveOpSpec(name=name, opcode=row, uops=uops,
                              rd1_en=_has_src1(spec)).sha(ver)
    op = dve_ops.DveOp(name, spec, subdim=False, uops_sha=shas)
    dve_ops.OPS.append(op)
    dve_ops.CUSTOM_DVE_SPECS[name] = spec
    return op


# ---------------------------------------------------------------------------
# device kernel
# ---------------------------------------------------------------------------
def _build_kernel(t_steps=T, variant=""):
    import os
    import concourse.bacc as bacc
    import concourse.mybir as mybir
    from concourse.tile import TileContext
    from concourse import masks

    OP = _get_add_max_op()
    f32 = mybir.dt.float32
    u8 = mybir.dt.uint8
    u32 = mybir.dt.uint32
    AX = mybir.AxisListType
    ALU = mybir.AluOpType
    ACT = mybir.ActivationFunctionType

    nc = bacc.Bacc()
    logit_d = nc.dram_tensor("logit", [BC, T, K], f32, kind="ExternalInput")
    trans_d = nc.dram_tensor("trans", [K, K], f32, kind="ExternalInput")
    start_d = nc.dram_tensor("start", [K], f32, kind="ExternalInput")
    lens_d = nc.dram_tensor("lens", [BC], f32, kind="ExternalInput")

    alphas_d = nc.dram_tensor("alphas", [t_steps, BC, K], f32, kind="ExternalOutput")
    alphaF_d = nc.dram_tensor("alphaF", [BC, K], f32, kind="ExternalOutput")
    NG = T // 8  # argmax groups of 8 timesteps across 128 partitions
    denom_d = nc.dram_tensor("denom", [128, NG], f32, kind="ExternalOutput")
    amax8_d = nc.dram_tensor("amax8", [128, NG, 8], f32, kind="ExternalOutput")
    argm8_d = nc.dram_tensor("argm8", [128, NG, 8], u32, kind="ExternalOutput")

    with TileContext(nc) as tc:
        with tc.tile_pool(name="st", bufs=1) as st, \
             tc.tile_pool(name="emitp", bufs=4) as emitp, \
             tc.tile_pool(name="vtp", bufs=2) as vtp, \
             tc.tile_pool(name="scp", bufs=2) as scp, \
             tc.tile_pool(name="fwd", bufs=2) as fwd, \
             tc.tile_pool(name="grp", bufs=3) as grp, \
             tc.tile_pool(name="pbc", bufs=4, space="PSUM") as pbc, \
             tc.tile_pool(name="ptp", bufs=3, space="PSUM") as ptp, \
             tc.tile_pool(name="psp", bufs=1, space="PSUM") as psp:

            # ---------------- static setup ----------------
            ident = st.tile([128, 128], f32, tag="ident", name="ident")
            masks.make_identity(nc, ident[:])
            ones1 = st.tile([1, 128], f32, tag="ones1", name="ones1")
            nc.vector.memset(ones1[:], 1.0)
            warm = st.tile([1, 8], f32, tag="warm", name="warm")
            nc.scalar.activation(warm[:], ones1[:, 0:8], ACT.Exp)
            nc.scalar.activation(warm[:], warm[:], ACT.Ln)

            Mrows = st.tile([128, 2, K], f32, tag="Mrows", name="Mrows")
            for h in range(2):
                nc.sync.dma_start(Mrows[:, h, :], trans_d[h * 128:(h + 1) * 128, :])
            MT = st.tile([128, 2, K], f32, tag="MT", name="MT")
            for ho in range(2):
                for hi in range(2):
                    blk = ptp.tile([128, 128], f32, tag="tp", name="blk")
                    nc.tensor.transpose(blk[:], Mrows[:, hi, ho * 128:(ho + 1) * 128], ident[:])
                    nc.scalar.copy(MT[:, ho, hi * 128:(hi + 1) * 128], blk[:])
            expM = st.tile([128, 2, K], f32, tag="expM", name="expM")
            nc.scalar.activation(expM[:, 0, :], Mrows[:, 0, :], ACT.Exp)
            nc.scalar.activation(expM[:, 1, :], Mrows[:, 1, :], ACT.Exp)

            lensT = st.tile([BC, 1], f32, tag="lensT", name="lensT")
            nc.sync.dma_start(lensT[:], lens_d[:].unsqueeze(1))
            # maskAll[b, t] = (t < len_b)  (uint8), built on gpsimd
            iotaT = st.tile([BC, T], mybir.dt.int32, tag="iotaT", name="iotaT")
            nc.gpsimd.iota(iotaT[:], pattern=[[1, T]], base=0, channel_multiplier=0)
            iotaF = st.tile([BC, T], f32, tag="iotaF", name="iotaF")
            nc.gpsimd.tensor_copy(iotaF[:], iotaT[:])
            maskAll = st.tile([BC, T], u8, tag="maskAll", name="maskAll")
            nc.vector.tensor_tensor(maskAll[:], iotaF[:],
                                    lensT[:, 0:1].to_broadcast((BC, T)), ALU.is_lt)
            maskF = st.tile([BC, T], f32, tag="maskF", name="maskF")
            nc.vector.tensor_tensor(maskF[:], iotaF[:],
                                    lensT[:, 0:1].to_broadcast((BC, T)), ALU.is_lt)
            maskFi = st.tile([BC, T], f32, tag="maskFi", name="maskFi")
            nc.vector.tensor_tensor(maskFi[:], iotaF[:],
                                    lensT[:, 0:1].to_broadcast((BC, T)), ALU.is_ge)

            # ---------------- t = 0 init ----------------
            startRep = st.tile([BC, K], f32, tag="startRep", name="startRep")
            nc.sync.dma_start(startRep[:], start_d[:].unsqueeze(0).to_broadcast((BC, K)))
            emit0 = emitp.tile([BC, K], f32, tag="emit", name="emit0")
            nc.sync.dma_start(emit0[:], logit_d[:, 0, :])
            alphaV = st.tile([BC, K], f32, tag="alphaV", name="alphaV")
            alphaF = st.tile([BC, K], f32, tag="alphaF", name="alphaF")
            nc.vector.tensor_tensor(alphaV[:], startRep[:], emit0[:], ALU.add)
            nc.scalar.copy(alphaF[:], alphaV[:])
            aFlat = st.tile([1, BC * K], f32, tag="aFlat", name="aFlat")
            nc.sync.dma_start(aFlat[:], alphaV[:])
            nc.gpsimd.dma_start(alphas_d[0], alphaV[:])

            # ---------------- main recursion ----------------
            for t in range(1, t_steps):
                emit = emitp.tile([BC, K], f32, tag="emit", name="emit")
                nc.sync.dma_start(emit[:], logit_d[:, t, :])
                mk1 = maskF[:, t:t + 1].to_broadcast((BC, K))
                mk0 = maskFi[:, t:t + 1].to_broadcast((BC, K))
                if not ("nofwd" in variant):
                    negm = fwd.tile([BC, 1], f32, tag="negm", name="negm")
                    nc.vector.tensor_reduce(negm[:], alphaF[:], axis=AX.X, op=ALU.max,
                                            negate=True)

                stale = "stale" in variant
                no_fwd = "nofwd" in variant
                # --- viterbi: VT[j, h, b] = max_i(alpha[b, i] + M[i, j]) ---
                VT = vtp.tile([128, 2, BC], f32, tag="VT", name="VT")
                for b in range(BC):
                    bc = pbc.tile([128, K], f32, tag="bcast", name="bc")
                    nc.tensor.matmul(bc[:], ones1[:], aFlat[:, b * K:(b + 1) * K],
                                     start=True, stop=True)
                    for h in range(2):
                        sc = scp.tile([128, K], f32, tag="sc", name="sc")
                        nc.vector._custom_dve(OP, out=sc[:], in0=MT[:, h, :],
                                              in1=bc[:], accum_out=VT[:, h, b:b + 1])
                VTt = ptp.tile([BC, K], f32, tag="tp", name="VTt")
                for h in range(2):
                    nc.tensor.transpose(VTt[:, h * 128:(h + 1) * 128], VT[:, h, :], ident[:])
                aVnew = scp.tile([BC, K], f32, tag="aVnew", name="aVnew")
                nc.vector.tensor_tensor(aVnew[:], VTt[:], emit[:], ALU.add)
                nc.vector.copy_predicated(alphaV[:], maskAll[:, t:t + 1].to_broadcast((BC, K)), aVnew[:])
                if not stale:
                    nc.sync.dma_start(aFlat[:], alphaV[:])
                nc.gpsimd.dma_start(alphas_d[t], alphaV[:])

                if no_fwd:
                    continue
                # --- forward: alphaF = log(exp(alphaF - m) @ expM) + m + emit ---
                Pt = fwd.tile([BC, K], f32, tag="Pt", name="Pt")
                nc.scalar.activation(Pt[:], alphaF[:], ACT.Exp, bias=negm[:])
                PTsb = fwd.tile([128, 2, BC], f32, tag="PTsb", name="PTsb")
                for h in range(2):
                    pth = ptp.tile([128, BC], f32, tag="tp", name="pth")
                    nc.tensor.transpose(pth[:], Pt[:, h * 128:(h + 1) * 128],
                                        ident[:BC, :BC])
                    nc.scalar.copy(PTsb[:, h, :], pth[:])
                S = psp.tile([BC, K], f32, tag="S", name="S")
                nc.tensor.matmul(S[:], PTsb[:, 0, :], expM[:, 0, :], start=True, stop=False)
                nc.tensor.matmul(S[:], PTsb[:, 1, :], expM[:, 1, :], start=False, stop=True)
                logS = fwd.tile([BC, K], f32, tag="logS", name="logS")
                nc.scalar.activation(logS[:], S[:], ACT.Ln)
                aFnew = fwd.tile([BC, K], f32, tag="aFnew", name="aFnew")
                nc.vector.scalar_tensor_tensor(aFnew[:], logS[:], negm[:], emit[:],
                                               op0=ALU.subtract, op1=ALU.add)
                sf1 = fwd.tile([BC, K], f32, tag="sf1", name="sf1")
                sf2 = fwd.tile([BC, K], f32, tag="sf2", name="sf2")
                nc.gpsimd.tensor_tensor(sf1[:], aFnew[:], mk1, ALU.mult)
                nc.gpsimd.tensor_tensor(sf2[:], alphaF[:], mk0, ALU.mult)
                nc.gpsimd.tensor_tensor(alphaF[:], sf1[:], sf2[:], ALU.add)

            nc.gpsimd.dma_start(alphaF_d[:], alphaF[:])

            # ---------------- log-softmax denominators + argmax ----------------
            # groups of 8 timesteps: partition p = b*8 + ts, t = g*8 + ts
            denomAll = st.tile([128, NG], f32, tag="denomAll", name="denomAll")
            mxAll = st.tile([128, NG, 8], f32, tag="mxAll", name="mxAll")
            ixAll = st.tile([128, NG, 8], u32, tag="ixAll", name="ixAll")
            for g in range(NG):
                lg = grp.tile([128, K], f32, tag="lg", name="lg")
                nc.sync.dma_start(lg[:], logit_d[:, g * 8:(g + 1) * 8, :])
                ex = grp.tile([128, K], f32, tag="ex", name="ex")
                se = grp.tile([128, 1], f32, tag="se", name="se")
                nc.scalar.activation(ex[:], lg[:], ACT.Exp, accum_out=se[:])
                nc.scalar.activation(denomAll[:, g:g + 1], se[:], ACT.Ln)
                nc.vector.max(mxAll[:, g, :], lg[:])
                nc.vector.max_index(ixAll[:, g, :], mxAll[:, g, :], lg[:])
            nc.gpsimd.dma_start(denom_d[:], denomAll[:])
            nc.gpsimd.dma_start(amax8_d[:], mxAll[:])
            nc.gpsimd.dma_start(argm8_d[:], ixAll[:])

    nc.compile()
    return nc


def _get_nc(t_steps=T, variant=""):
    key = ("nc", t_steps, variant)
    if key not in _CACHE:
        _CACHE[key] = _build_kernel(t_steps, variant)
    return _CACHE[key]


# ---------------------------------------------------------------------------
# host orchestration
# ---------------------------------------------------------------------------
def kernel(logit, transitions, start_transitions, end_transitions, target, seq_lens,
           t_steps=T, profile=False):
    from concourse.bass_utils import run_bass_kernel_spmd
    run_kwargs = dict(trace=True) if profile else {}

    logit = np.ascontiguousarray(logit, dtype=np.float32)
    trans = np.ascontiguousarray(transitions, dtype=np.float32)
    start = np.ascontiguousarray(start_transitions, dtype=np.float32)
    end = np.ascontiguousarray(end_transitions, dtype=np.float32)
    tgt = np.asarray(target)
    lens = np.asarray(seq_lens)
    lens_i = lens.astype(np.int64)
    lens_f = lens.astype(np.float32)

    import os
    nc = _get_nc(t_steps, os.environ.get("CRF_VARIANT", ""))
    in_maps = []
    for c in range(NCORES):
        sl = slice(c * BC, (c + 1) * BC)
        in_maps.append(dict(logit=logit[sl], trans=trans, start=start,
                            lens=lens_f[sl]))
    import time as _time
    _t0 = _time.time()
    res = None
    for attempt in range(3):
        try:
            res = run_bass_kernel_spmd(nc, in_maps, list(range(NCORES)), **run_kwargs)
            break
        except Exception as e:  # transient NRT device wedge: cool down, retry
            msg = str(e)
            if attempt < 2 and ("UNAVAILABLE" in msg or "unrecoverable" in msg
                                or "INTERNAL" in msg):
                _time.sleep(30)
                continue
            raise
    outs = res.results
    global LAST_SPMD_WALL
    LAST_SPMD_WALL = _time.time() - _t0

    alphas = np.concatenate([o["alphas"] for o in outs], axis=1)  # [t, B, K]
    alphaF = np.concatenate([o["alphaF"] for o in outs], axis=0)  # [B, K]
    NG = T // 8

    def ungroup(arrs, dtype):
        # [128, NG] with partition p = b*8 + ts -> [BC, T] per core (t = g*8+ts)
        parts = []
        for a in arrs:
            a = a.reshape(BC, 8, NG)                # [b, ts, g]
            parts.append(a.transpose(0, 2, 1).reshape(BC, NG * 8))
        return np.concatenate(parts, axis=0).astype(dtype)  # [B, T]

    denom = ungroup([o["denom"] for o in outs], np.float32)
    amax = ungroup([o["amax8"][:, :, 0] for o in outs], np.float32)
    argm = np.clip(ungroup([o["argm8"][:, :, 0] for o in outs], np.int64), 0, K - 1)

    bidx = np.arange(B)

    # ---- loss (numerator on host, logZ from device forward alphas) ----
    mask = (np.arange(T)[None, :] < lens_i[:, None])
    mf = mask.astype(np.float32)
    t0 = tgt[:, 0]
    score = start[t0].astype(np.float32) + logit[bidx, 0, t0]
    trans_sc = trans[tgt[:, :-1], tgt[:, 1:]]                  # [B, T-1]
    emit_sc = np.take_along_axis(logit, tgt[..., None], axis=2)[..., 0]
    score = score + np.sum((trans_sc + emit_sc[:, 1:]) * mf[:, 1:], axis=1,
                           dtype=np.float32)
    last_tags = tgt[bidx, np.clip(lens_i - 1, -T, T - 1)]
    score = score + end[last_tags]
    aT_e = alphaF + end[None, :]
    m = aT_e.max(axis=1)
    logZ = np.log(np.exp(aT_e - m[:, None]).sum(axis=1)) + m
    ll = score - logZ
    loss = np.float32(np.mean(-ll.astype(np.float32)))

    # ---- viterbi backtrace (bit-exact; lowest-index tie-break) ----
    transT = trans.T                                          # [K(j), K(i)]
    last_alpha = alphas[t_steps - 1] + end[None, :]
    best_last = np.argmax(last_alpha, axis=1)
    pred = np.empty((B, T), dtype=np.int32)
    pred[:, t_steps - 1:] = best_last[:, None]
    tag = best_last
    for t in range(t_steps - 1, 0, -1):
        cand = alphas[t - 1] + transT[tag]                    # [B, K]
        prev = np.argmax(cand, axis=1)
        live = mask[:, t]
        tag = np.where(live, prev, tag)
        pred[:, t - 1] = tag
    pred_full = pred

    # ---- log_probs: swap + subtract denominators ----
    out = logit.copy()
    ti = np.arange(T)[None, :]
    bi = bidx[:, None]
    labv = np.take_along_axis(logit, pred_full[..., None].astype(np.int64),
                              axis=2)[..., 0]
    out[bi, ti, pred_full] = amax
    out[bi, ti, argm] = labv
    log_probs = out - denom[..., None]

    if profile:
        return (loss, pred_full, log_probs), res
    return loss, pred_full, log_probs


if __name__ == "__main__":
    import reference
    inputs = {k: np.asarray(v) for k, v in reference.setup_inputs().items()}
    got = kernel(**inputs)
    exp = [np.asarray(x) for x in reference.reference(**inputs)]
    for name, g, e in zip(("loss", "pred", "log_probs"), got, exp):
        g = np.asarray(g)
        err = np.abs(g.astype(np.float64) - e.astype(np.float64))
        rel = err.max() / max(np.abs(e).max(), 1e-9)
        print(f"{name}: max abs {err.max():.3e} rel {rel:.3e} "
              f"exact={np.array_equal(g, e)}")


# revision 17
# speedup vs baseline: 1.1192x; 1.1192x over previous
"""CRF output layer (loss + viterbi pred + rearranged log_probs) on 8 Trainium2 cores.

Device (per core, 16 of 128 batch rows):
  - Viterbi value recursion: per step, PE broadcasts each row's alpha across
    partitions (ones-matmul into PSUM); a custom fused DVE op computes
    max_i(alpha_i + M_ij) per j in one pass (bit-exact fp32). Alphas for every
    step stream to HBM.
  - Forward (logsumexp) recursion in exp space: P = exp(alpha - rowmax) via ACT,
    fp32 matmul with exp(M) on PE, Ln + add-back on ACT/DVE.
  - log-softmax denominators + per-token argmax/max of logits.
Host: gold-path score, backtrace from stored alphas (bit-exact, lowest-index
tie-break), loss assembly, logit swap + denominator subtraction.
"""
import numpy as np

B, T, K = 128, 512, 256
NCORES = 8
BC = B // NCORES  # 16 rows per core

_CACHE = {}
LAST_SPMD_WALL = None


# ---------------------------------------------------------------------------
# custom DVE op: out = in0 + in1 ; accum_out = max over free dim (seed -FLT_MAX)
# ---------------------------------------------------------------------------
def _get_add_max_op():
    import concourse.dve_ops as dve_ops
    from concourse.dve_spec import Spec, Src0, Src1, maxx, lower, _has_src1
    from concourse.dve_uop import DveOpSpec

    name = "ANT_ADD_MAX_REDUCE_X"
    for op in dve_ops.OPS:
        if op.name == name:
            return op
    spec = Spec(body=Src0 + Src1, accum=maxx)
    row = dve_ops._CUSTOM_DVE_ROW_BASE + len(dve_ops.OPS)
    assert row < 0x20
    dve_ops._SUB_OPCODE_FOR_NAME[name] = row
    shas = {}
    for ver in ("v3", "v4"):
        uops = lower(spec, ver=ver)
        shas[ver] = DveOpSpec(name=name, opcode=row, uops=uops,
                              rd1_en=_has_src1(spec)).sha(ver)
    op = dve_ops.DveOp(name, spec, subdim=False, uops_sha=shas)
    dve_ops.OPS.append(op)
    dve_ops.CUSTOM_DVE_SPECS[name] = spec
    return op


# ---------------------------------------------------------------------------
# device kernel
# ---------------------------------------------------------------------------
def _build_kernel(t_steps=T, variant=""):
    import os
    import concourse.bacc as bacc
    import concourse.mybir as mybir
    from concourse.tile import TileContext
    from concourse import masks

    OP = _get_add_max_op()
    f32 = mybir.dt.float32
    u8 = mybir.dt.uint8
    u32 = mybir.dt.uint32
    AX = mybir.AxisListType
    ALU = mybir.AluOpType
    ACT = mybir.ActivationFunctionType

    nc = bacc.Bacc()
    logit_d = nc.dram_tensor("logit", [BC, T, K], f32, kind="ExternalInput")
    trans_d = nc.dram_tensor("trans", [K, K], f32, kind="ExternalInput")
    start_d = nc.dram_tensor("start", [K], f32, kind="ExternalInput")
    lens_d = nc.dram_tensor("lens", [BC], f32, kind="ExternalInput")

    alphas_d = nc.dram_tensor("alphas", [t_steps, BC, K], f32, kind="ExternalOutput")
    alphaF_d = nc.dram_tensor("alphaF", [BC, K], f32, kind="ExternalOutput")
    NG = T // 8  # argmax groups of 8 timesteps across 128 partitions
    denom_d = nc.dram_tensor("denom", [128, NG], f32, kind="ExternalOutput")
    amax8_d = nc.dram_tensor("amax8", [128, NG, 8], f32, kind="ExternalOutput")
    argm8_d = nc.dram_tensor("argm8", [128, NG, 8], u32, kind="ExternalOutput")

    with TileContext(nc) as tc:
        with tc.tile_pool(name="st", bufs=1) as st, \
             tc.tile_pool(name="emitp", bufs=4) as emitp, \
             tc.tile_pool(name="vtp", bufs=2) as vtp, \
             tc.tile_pool(name="scp", bufs=2) as scp, \
             tc.tile_pool(name="fwd", bufs=2) as fwd, \
             tc.tile_pool(name="grp", bufs=3) as grp, \
             tc.tile_pool(name="pbc", bufs=4, space="PSUM") as pbc, \
             tc.tile_pool(name="ptp", bufs=3, space="PSUM") as ptp, \
             tc.tile_pool(name="psp", bufs=1, space="PSUM") as psp:

            # ---------------- static setup ----------------
            ident = st.tile([128, 128], f32, tag="ident", name="ident")
            masks.make_identity(nc, ident[:])
            ones1 = st.tile([1, 128], f32, tag="ones1", name="ones1")
            nc.vector.memset(ones1[:], 1.0)
            warm = st.tile([1, 8], f32, tag="warm", name="warm")
            nc.scalar.activation(warm[:], ones1[:, 0:8], ACT.Exp)
            nc.scalar.activation(warm[:], warm[:], ACT.Ln)

            Mrows = st.tile([128, 2, K], f32, tag="Mrows", name="Mrows")
            for h in range(2):
                nc.sync.dma_start(Mrows[:, h, :], trans_d[h * 128:(h + 1) * 128, :])
            MT = st.tile([128, 2, K], f32, tag="MT", name="MT")
            for ho in range(2):
                for hi in range(2):
                    blk = ptp.tile([128, 128], f32, tag="tp", name="blk")
                    nc.tensor.transpose(blk[:], Mrows[:, hi, ho * 128:(ho + 1) * 128], ident[:])
                    nc.scalar.copy(MT[:, ho, hi * 128:(hi + 1) * 128], blk[:])
            expM = st.tile([128, 2, K], f32, tag="expM", name="expM")
            nc.scalar.activation(expM[:, 0, :], Mrows[:, 0, :], ACT.Exp)
            nc.scalar.activation(expM[:, 1, :], Mrows[:, 1, :], ACT.Exp)

            lensT = st.tile([BC, 1], f32, tag="lensT", name="lensT")
            nc.sync.dma_start(lensT[:], lens_d[:].unsqueeze(1))
            # maskAll[b, t] = (t < len_b)  (uint8), built on gpsimd
            iotaT = st.tile([BC, T], mybir.dt.int32, tag="iotaT", name="iotaT")
            nc.gpsimd.iota(iotaT[:], pattern=[[1, T]], base=0, channel_multiplier=0)
            iotaF = st.tile([BC, T], f32, tag="iotaF", name="iotaF")
            nc.gpsimd.tensor_copy(iotaF[:], iotaT[:])
            maskAll = st.tile([BC, T], u8, tag="maskAll", name="maskAll")
            nc.vector.tensor_tensor(maskAll[:], iotaF[:],
                                    lensT[:, 0:1].to_broadcast((BC, T)), ALU.is_lt)
            maskF = st.tile([BC, T], f32, tag="maskF", name="maskF")
            nc.vector.tensor_tensor(maskF[:], iotaF[:],
                                    lensT[:, 0:1].to_broadcast((BC, T)), ALU.is_lt)
            maskFi = st.tile([BC, T], f32, tag="maskFi", name="maskFi")
            nc.vector.tensor_tensor(maskFi[:], iotaF[:],
                                    lensT[:, 0:1].to_broadcast((BC, T)), ALU.is_ge)

            # ---------------- t = 0 init ----------------
            startRep = st.tile([BC, K], f32, tag="startRep", name="startRep")
            nc.sync.dma_start(startRep[:], start_d[:].unsqueeze(0).to_broadcast((BC, K)))
            emit0 = emitp.tile([BC, K], f32, tag="emit", name="emit0")
            nc.sync.dma_start(emit0[:], logit_d[:, 0, :])
            alphaV = st.tile([BC, K], f32, tag="alphaV", name="alphaV")
            alphaF = st.tile([BC, K], f32, tag="alphaF", name="alphaF")
            nc.vector.tensor_tensor(alphaV[:], startRep[:], emit0[:], ALU.add)
            nc.scalar.copy(alphaF[:], alphaV[:])
            aFlat = st.tile([1, BC * K], f32, tag="aFlat", name="aFlat")
            nc.sync.dma_start(aFlat[:], alphaV[:])
            nc.gpsimd.dma_start(alphas_d[0], alphaV[:])

            # ---------------- main recursion ----------------
            for t in range(1, t_steps):
                emit = emitp.tile([BC, K], f32, tag="emit", name="emit")
                nc.sync.dma_start(emit[:], logit_d[:, t, :])
                mk1 = maskF[:, t:t + 1].to_broadcast((BC, K))
                mk0 = maskFi[:, t:t + 1].to_broadcast((BC, K))
                stale = "stale" in variant
                no_fwd = "nofwd" in variant
                # --- viterbi: VT[j, h, b] = max_i(alpha[b, i] + M[i, j]) ---
                VT = vtp.tile([128, 2, BC], f32, tag="VT", name="VT")
                for b in range(BC):
                    bc = pbc.tile([128, K], f32, tag="bcast", name="bc")
                    nc.tensor.matmul(bc[:], ones1[:], aFlat[:, b * K:(b + 1) * K],
                                     start=True, stop=True)
                    for h in range(2):
                        sc = scp.tile([128, K], f32, tag="sc", name="sc")
                        nc.vector._custom_dve(OP, out=sc[:], in0=MT[:, h, :],
                                              in1=bc[:], accum_out=VT[:, h, b:b + 1])
                VTt = ptp.tile([BC, K], f32, tag="tp", name="VTt")
                for h in range(2):
                    nc.tensor.transpose(VTt[:, h * 128:(h + 1) * 128], VT[:, h, :], ident[:])
                aVnew = scp.tile([BC, K], f32, tag="aVnew", name="aVnew")
                nc.vector.tensor_tensor(aVnew[:], VTt[:], emit[:], ALU.add)
                nc.vector.copy_predicated(alphaV[:], maskAll[:, t:t + 1].to_broadcast((BC, K)), aVnew[:])
                if not stale:
                    nc.sync.dma_start(aFlat[:], alphaV[:])
                nc.gpsimd.dma_start(alphas_d[t], alphaV[:])

                if no_fwd:
                    continue
                # --- forward: alphaF = log(exp(alphaF - m) @ expM) + m + emit ---
                negm = fwd.tile([BC, 1], f32, tag="negm", name="negm")
                nc.vector.tensor_reduce(negm[:], alphaF[:], axis=AX.X, op=ALU.max,
                                        negate=True)
                Pt = fwd.tile([BC, K], f32, tag="Pt", name="Pt")
                nc.scalar.activation(Pt[:], alphaF[:], ACT.Exp, bias=negm[:])
                PTsb = fwd.tile([128, 2, BC], f32, tag="PTsb", name="PTsb")
                for h in range(2):
                    pth = ptp.tile([128, BC], f32, tag="tp", name="pth")
                    nc.tensor.transpose(pth[:], Pt[:, h * 128:(h + 1) * 128],
                                        ident[:BC, :BC])
                    nc.scalar.copy(PTsb[:, h, :], pth[:])
                S = psp.tile([BC, K], f32, tag="S", name="S")
                nc.tensor.matmul(S[:], PTsb[:, 0, :], expM[:, 0, :], start=True, stop=False)
                nc.tensor.matmul(S[:], PTsb[:, 1, :], expM[:, 1, :], start=False, stop=True)
                logS = fwd.tile([BC, K], f32, tag="logS", name="logS")
                nc.scalar.activation(logS[:], S[:], ACT.Ln)
                aFnew = fwd.tile([BC, K], f32, tag="aFnew", name="aFnew")
                nc.vector.scalar_tensor_tensor(aFnew[:], logS[:], negm[:], emit[:],
                                               op0=ALU.subtract, op1=ALU.add)
                sf1 = fwd.tile([BC, K], f32, tag="sf1", name="sf1")
                sf2 = fwd.tile([BC, K], f32, tag="sf2", name="sf2")
                nc.gpsimd.tensor_tensor(sf1[:], aFnew[:], mk1, ALU.mult)
                nc.gpsimd.tensor_tensor(sf2[:], alphaF[:], mk0, ALU.mult)
                nc.gpsimd.tensor_tensor(alphaF[:], sf1[:], sf2[:], ALU.add)

            nc.gpsimd.dma_start(alphaF_d[:], alphaF[:])

            # ---------------- log-softmax denominators + argmax ----------------
            # groups of 8 timesteps: partition p = b*8 + ts, t = g*8 + ts
            denomAll = st.tile([128, NG], f32, tag="denomAll", name="denomAll")
            mxAll = st.tile([128, NG, 8], f32, tag="mxAll", name="mxAll")
            ixAll = st.tile([128, NG, 8], u32, tag="ixAll", name="ixAll")
            for g in range(NG):
                lg = grp.tile([128, K], f32, tag="lg", name="lg")
                nc.sync.dma_start(lg[:], logit_d[:, g * 8:(g + 1) * 8, :])
                ex = grp.tile([128, K], f32, tag="ex", name="ex")
                se = grp.tile([128, 1], f32, tag="se", name="se")
                nc.scalar.activation(ex[:], lg[:], ACT.Exp, accum_out=se[:])
                nc.scalar.activation(denomAll[:, g:g + 1], se[:], ACT.Ln)
                nc.vector.max(mxAll[:, g, :], lg[:])
                nc.vector.max_index(ixAll[:, g, :], mxAll[:, g, :], lg[:])
            nc.gpsimd.dma_start(denom_d[:], denomAll[:])
            nc.gpsimd.dma_start(amax8_d[:], mxAll[:])
            nc.gpsimd.dma_start(argm8_d[:], ixAll[:])

    nc.compile()
    return nc


def _get_nc(t_steps=T, variant=""):
    key = ("nc", t_steps, variant)
    if key not in _CACHE:
        _CACHE[key] = _build_kernel(t_steps, variant)
    return _CACHE[key]


# ---------------------------------------------------------------------------
# host orchestration
# ---------------------------------------------------------------------------
def kernel(logit, transitions, start_transitions, end_transitions, target, seq_lens,
           t_steps=T, profile=False):
    from concourse.bass_utils import run_bass_kernel_spmd
    run_kwargs = dict(trace=True) if profile else {}

    logit = np.ascontiguousarray(logit, dtype=np.float32)
    trans = np.ascontiguousarray(transitions, dtype=np.float32)
    start = np.ascontiguousarray(start_transitions, dtype=np.float32)
    end = np.ascontiguousarray(end_transitions, dtype=np.float32)
    tgt = np.asarray(target)
    lens = np.asarray(seq_lens)
    lens_i = lens.astype(np.int64)
    lens_f = lens.astype(np.float32)

    import os
    nc = _get_nc(t_steps, os.environ.get("CRF_VARIANT", ""))
    in_maps = []
    for c in range(NCORES):
        sl = slice(c * BC, (c + 1) * BC)
        in_maps.append(dict(logit=logit[sl], trans=trans, start=start,
                            lens=lens_f[sl]))
    import time as _time
    _t0 = _time.time()
    res = None
    for attempt in range(3):
        try:
            res = run_bass_kernel_spmd(nc, in_maps, list(range(NCORES)), **run_kwargs)
            break
        except Exception as e:  # transient NRT device wedge: cool down, retry
            msg = str(e)
            if attempt < 2 and ("UNAVAILABLE" in msg or "unrecoverable" in msg
                                or "INTERNAL" in msg):
                _time.sleep(30)
                continue
            raise
    outs = res.results
    global LAST_SPMD_WALL
    LAST_SPMD_WALL = _time.time() - _t0

    alphas = np.concatenate([o["alphas"] for o in outs], axis=1)  # [t, B, K]
    alphaF = np.concatenate([o["alphaF"] for o in outs], axis=0)  # [B, K]
    NG = T // 8

    def ungroup(arrs, dtype):
        # [128, NG] with partition p = b*8 + ts -> [BC, T] per core (t = g*8+ts)
        parts = []
        for a in arrs:
            a = a.reshape(BC, 8, NG)                # [b, ts, g]
            parts.append(a.transpose(0, 2, 1).reshape(BC, NG * 8))
        return np.concatenate(parts, axis=0).astype(dtype)  # [B, T]

    denom = ungroup([o["denom"] for o in outs], np.float32)
    amax = ungroup([o["amax8"][:, :, 0] for o in outs], np.float32)
    argm = np.clip(ungroup([o["argm8"][:, :, 0] for o in outs], np.int64), 0, K - 1)

    bidx = np.arange(B)

    # ---- loss (numerator on host, logZ from device forward alphas) ----
    mask = (np.arange(T)[None, :] < lens_i[:, None])
    mf = mask.astype(np.float32)
    t0 = tgt[:, 0]
    score = start[t0].astype(np.float32) + logit[bidx, 0, t0]
    trans_sc = trans[tgt[:, :-1], tgt[:, 1:]]                  # [B, T-1]
    emit_sc = np.take_along_axis(logit, tgt[..., None], axis=2)[..., 0]
    score = score + np.sum((trans_sc + emit_sc[:, 1:]) * mf[:, 1:], axis=1,
                           dtype=np.float32)
    last_tags = tgt[bidx, np.clip(lens_i - 1, -T, T - 1)]
    score = score + end[last_tags]
    aT_e = alphaF + end[None, :]
    m = aT_e.max(axis=1)
    logZ = np.log(np.exp(aT_e - m[:, None]).sum(axis=1)) + m
    ll = score - logZ
    loss = np.float32(np.mean(-ll.astype(np.float32)))

    # ---- viterbi backtrace (bit-exact; lowest-index tie-break) ----
    transT = trans.T                                          # [K(j), K(i)]
    last_alpha = alphas[t_steps - 1] + end[None, :]
    best_last = np.argmax(last_alpha, axis=1)
    pred = np.empty((B, T), dtype=np.int32)
    pred[:, t_steps - 1:] = best_last[:, None]
    tag = best_last
    for t in range(t_steps - 1, 0, -1):
        cand = alphas[t - 1] + transT[tag]                    # [B, K]
        prev = np.argmax(cand, axis=1)
        live = mask[:, t]
        tag = np.where(live, prev, tag)
        pred[:, t - 1] = tag
    pred_full = pred

    # ---- log_probs: swap + subtract denominators ----
    out = logit.copy()
    ti = np.arange(T)[None, :]
    bi = bidx[:, None]
    labv = np.take_along_axis(logit, pred_full[..., None].astype(np.int64),
                              axis=2)[..., 0]
    out[bi, ti, pred_full] = amax
    out[bi, ti, argm] = labv
    log_probs = out - denom[..., None]

    if profile:
        return (loss, pred_full, log_probs), res
    return loss, pred_full, log_probs


if __name__ == "__main__":
    import reference
    inputs = {k: np.asarray(v) for k, v in reference.setup_inputs().items()}
    got = kernel(**inputs)
    exp = [np.asarray(x) for x in reference.reference(**inputs)]
    for name, g, e in zip(("loss", "pred", "log_probs"), got, exp):
        g = np.asarray(g)
        err = np.abs(g.astype(np.float64) - e.astype(np.float64))
        rel = err.max() / max(np.abs(e).max(), 1e-9)
        print(f"{name}: max abs {err.max():.3e} rel {rel:.3e} "
              f"exact={np.array_equal(g, e)}")
